# Initial kernel scaffold
#
import sys

if "/opt/trn_rl_repo" not in sys.path:
    sys.path.insert(0, "/opt/trn_rl_repo")

import numpy as np
import concourse.bass as bass
import concourse.bacc as bacc
import concourse.mybir as mybir
import concourse.tile as tile
from concourse import bass_utils
import ml_dtypes
jnp_bf16 = ml_dtypes.bfloat16

B, T, I, H, C = 512, 1024, 64, 128, 10
NCORES = 8
BL = B // NCORES          # batch per core
CH = 128                  # timesteps per DMA chunk
GS = 8                    # steps per psum group (8*64 = 512 = one bank)
FP32 = mybir.dt.float32
BF16 = mybir.dt.bfloat16

_cache = {}


def _build():
    nc = bacc.Bacc("TRN2", debug=False, num_devices=NCORES)
    xt_d = nc.dram_tensor("xt", [I + 1, T * BL], BF16, kind="ExternalInput")
    wx_d = nc.dram_tensor("wx", [I + 1, 4 * H], BF16, kind="ExternalInput")
    wh_d = nc.dram_tensor("wh", [H, 4 * H], BF16, kind="ExternalInput")
    wfc_d = nc.dram_tensor("wfc", [H, C], BF16, kind="ExternalInput")
    bfc_d = nc.dram_tensor("bfcb", [BL, C], FP32, kind="ExternalInput")
    y_d = nc.dram_tensor("y", [BL, C], FP32, kind="ExternalOutput")

    with tile.TileContext(nc) as tc:
        with (
            tc.tile_pool(name="const", bufs=1) as cpool,
            tc.tile_pool(name="xch", bufs=2) as xpool,
            tc.tile_pool(name="act", bufs=3) as apool,
            tc.tile_pool(name="ps", bufs=2, space="PSUM") as ppool,
        ):
            wx_s = cpool.tile([I + 1, 4 * H], BF16)
            wh_s = cpool.tile([H, 4 * H], BF16)
            wfc_s = cpool.tile([H, C], BF16)
            bfc_s = cpool.tile([BL, C], FP32)
            h = cpool.tile([H, BL], BF16)
            c = cpool.tile([H, BL], FP32)

            nc.sync.dma_start(wx_s[:], wx_d.ap())
            nc.sync.dma_start(wh_s[:], wh_d.ap())
            nc.sync.dma_start(wfc_s[:], wfc_d.ap())
            nc.sync.dma_start(bfc_s[:], bfc_d.ap())
            nc.vector.memset(h[:], 0.0)
            nc.vector.memset(c[:], 0.0)

            xc = None
            for grp in range(T // GS):
                if grp % (CH // GS) == 0:
                    ci = grp // (CH // GS)
                    xc = xpool.tile([I + 1, CH * BL], BF16)
                    nc.sync.dma_start(
                        xc[:], xt_d.ap()[:, ci * CH * BL : (ci + 1) * CH * BL]
                    )
                off = (grp % (CH // GS)) * GS * BL

                ps = ppool.tile([128, 4 * GS * BL], FP32, tag="ps")
                psr = ps.rearrange("p (g n) -> p g n", g=4)
                for g in range(4):
                    nc.tensor.matmul(
                        psr[:, g, :],
                        wx_s[:, g * H : (g + 1) * H],
                        xc[:, off : off + GS * BL],
                        start=True,
                        stop=False,
                    )
                for k in range(GS):
                    col = slice(k * BL, (k + 1) * BL)
                    for g in range(4):
                        nc.tensor.matmul(
                            psr[:, g, col],
                            wh_s[:, g * H : (g + 1) * H],
                            h[:],
                            start=False,
                            stop=(k == GS - 1),
                        )
                    s = apool.tile([128, 4 * BL], FP32, tag="s")
                    sr = s.rearrange("p (g n) -> p g n", g=4)
                    # gates order: f, o, i, g
                    nc.scalar.activation(
                        sr[:, 0:3, :],
                        psr[:, 0:3, col],
                        mybir.ActivationFunctionType.Sigmoid,
                    )
                    nc.scalar.activation(
                        sr[:, 3, :],
                        psr[:, 3, col],
                        mybir.ActivationFunctionType.Tanh,
                    )
                    ig = apool.tile([H, BL], FP32, tag="ig")
                    cf = apool.tile([H, BL], FP32, tag="cf")
                    nc.vector.tensor_mul(ig[:], sr[:, 2, :], sr[:, 3, :])
                    nc.vector.tensor_mul(cf[:], c[:], sr[:, 0, :])
                    nc.vector.tensor_add(c[:], cf[:], ig[:])
                    nc.vector.tensor_mul(h[:], c[:], sr[:, 1, :])

            ypt = ppool.tile([128, 4 * GS * BL], FP32, tag="ps")
            yp = ypt[:BL, :C]
            nc.tensor.matmul(yp, h[:, :], wfc_s[:], start=True, stop=True)
            y_s = cpool.tile([BL, C], FP32)
            nc.vector.tensor_add(y_s[:], yp, bfc_s[:])
            nc.sync.dma_start(y_d.ap(), y_s[:])

    nc.compile()
    return nc


def kernel(x, Wf, bf, Wo, bo, Wi, bi, Wg, bg, Wfc, bfc):
    if "nc" not in _cache:
        _cache["nc"] = _build()
    nc = _cache["nc"]

    gates = [(Wf, bf), (Wo, bo), (Wi, bi), (Wg, bg)]  # f, o, i, g
    wx = np.concatenate(
        [
            np.concatenate([W[:, :I].T, b[None, :]], axis=0).astype(np.float32)
            for W, b in gates
        ],
        axis=1,
    ).astype(jnp_bf16)  # [I+1, 4H]
    wh = np.concatenate([W[:, I:].T for W, _ in gates], axis=1).astype(
        np.float32
    ).astype(jnp_bf16)  # [H, 4H]
    wfc = np.ascontiguousarray(Wfc.T).astype(jnp_bf16)  # [H, C]
    bfcb = np.broadcast_to(bfc, (BL, C)).astype(np.float32).copy()

    in_maps = []
    for cidx in range(NCORES):
        xs = np.asarray(x[cidx * BL : (cidx + 1) * BL], np.float32)  # [BL,T,I]
        xt = np.ascontiguousarray(xs.transpose(2, 1, 0)).reshape(I, T * BL)
        xt = np.concatenate([xt, np.ones((1, T * BL), np.float32)], axis=0).astype(jnp_bf16)
        in_maps.append(
            {"xt": xt, "wx": wx, "wh": wh, "wfc": wfc, "bfcb": bfcb}
        )

    _cache["in_maps"] = in_maps
    res = bass_utils.run_bass_kernel_spmd(
        nc, in_maps, core_ids=list(range(NCORES))
    )
    return np.concatenate([r["y"] for r in res.results], axis=0)



# revision 2
# speedup vs baseline: 1.2135x; 1.2135x over previous
import sys

if "/opt/trn_rl_repo" not in sys.path:
    sys.path.insert(0, "/opt/trn_rl_repo")

import numpy as np
import concourse.bass as bass
import concourse.bacc as bacc
import concourse.mybir as mybir
import concourse.tile as tile
from concourse import bass_utils
import ml_dtypes

jnp_bf16 = ml_dtypes.bfloat16

B, T, I, H, C = 512, 1024, 64, 128, 10
NCORES = 8
BL = B // NCORES          # batch per core (64)
HB = BL // 2              # half-batch (32)
GS = 8                    # steps per psum group
CH = 128                  # timesteps per x DMA chunk
FP32 = mybir.dt.float32
BF16 = mybir.dt.bfloat16
ALU = mybir.AluOpType

_cache = {}

# Two phase-shifted half-batches (A = batch 0:32, B = 32:64). Per half a
# psum group tile [128, 4*GS*32] with col = g*(GS*32) + s*32 + b, so every
# matmul dst stays inside one bank and the halves use disjoint bank sets.
#
# W region layout per half (bf16, [128, 160]):
#   cols 0:32    c~ state  (c/2 + 1/2)
#   cols 32:64   sig(2*g_pre)      } one sigmoid ACT per half-step
#   cols 64:96   sig(f_pre)        } psum gate order: (g, f, i, o)
#   cols 96:128  sig(i_pre)        }
#   cols 128:160 sig(o_pre)        }
# stt1 (DVE): [r|q] = ([c~|s2g] - 0.5) * [sf|si]
# stt2 (DVE): c~    = (r + 0.5) + q
# stt3 (DVE): hh    = (c~ - 0.5) * so      (hh = h/2; Wh,Wfc pre-doubled)


def _build():
    nc = bacc.Bacc("TRN2", debug=False, num_devices=NCORES)
    xt_d = nc.dram_tensor("xt", [I + 1, T * BL], BF16, kind="ExternalInput")
    wx_d = nc.dram_tensor("wx", [I + 1, 4 * H], BF16, kind="ExternalInput")
    wh_d = nc.dram_tensor("wh", [H, 4 * H], BF16, kind="ExternalInput")
    wfc_d = nc.dram_tensor("wfc", [H, C], BF16, kind="ExternalInput")
    bfc_d = nc.dram_tensor("bfcb", [BL, C], FP32, kind="ExternalInput")
    y_d = nc.dram_tensor("y", [BL, C], FP32, kind="ExternalOutput")

    GC = GS * HB  # columns per gate per half-group (256)

    with tile.TileContext(nc) as tc:
        with (
            tc.tile_pool(name="const", bufs=1) as cpool,
            tc.tile_pool(name="xch", bufs=2) as xpool,
            tc.tile_pool(name="psA", bufs=2, space="PSUM") as ppoolA,
            tc.tile_pool(name="psB", bufs=2, space="PSUM") as ppoolB,
        ):
            wx_s = cpool.tile([I + 1, 4 * H], BF16)
            wh_s = cpool.tile([H, 4 * H], BF16)
            wfc_s = cpool.tile([H, C], BF16)
            bfc_s = cpool.tile([BL, C], FP32)
            W = [cpool.tile([128, 160], BF16, name=f"W{h}") for h in range(2)]
            X = [cpool.tile([128, 2 * HB], BF16, name=f"X{h}") for h in range(2)]
            hh = cpool.tile([H, BL], BF16)

            nc.sync.dma_start(wx_s[:], wx_d.ap())
            nc.sync.dma_start(wh_s[:], wh_d.ap())
            nc.sync.dma_start(wfc_s[:], wfc_d.ap())
            nc.sync.dma_start(bfc_s[:], bfc_d.ap())
            for h in range(2):
                nc.vector.memset(W[h][:, 0:32], 0.5)
            nc.vector.memset(hh[:], 0.0)

            pools = [ppoolA, ppoolB]
            NG = T // GS

            def alloc_group():
                return [
                    pools[h].tile([128, 4 * GC], FP32, tag="ps", name=f"ps{h}")
                    for h in range(2)
                ]

            def emit_xproj_one(grp, xc, P, k):
                # one x-projection matmul (gate g of half h), inside one
                # psum bank.  start=True on the first writer of each bank
                # (gate 0 -> bank 0, gate 2 -> bank 1 of the tile).
                xcv = xc.rearrange("p (t n) -> p t n", n=BL)
                t0 = (grp % (CH // GS)) * GS
                h, g = k // 4, k % 4
                nc.tensor.matmul(
                    P[h][:, g * GC : (g + 1) * GC],
                    wx_s[:, g * H : (g + 1) * H],
                    xcv[:, t0 : t0 + GS, h * HB : (h + 1) * HB],
                    start=(g % 2 == 0),
                    stop=False,
                )

            def fetch_chunk(grp):
                ci = grp // (CH // GS)
                xc = xpool.tile([I + 1, CH * BL], BF16)
                nc.sync.dma_start(
                    xc[:], xt_d.ap()[:, ci * CH * BL : (ci + 1) * CH * BL]
                )
                return xc

            xc = fetch_chunk(0)
            P = alloc_group()
            for k in range(8):
                emit_xproj_one(0, xc, P, k)
            for grp in range(NG):
                P_next = None
                for s in range(GS):
                    if grp + 1 < NG:
                        # scatter next group's x-projection matmuls (and
                        # chunk DMA), one per step, so each fits into PE
                        # idle windows instead of forming one long block
                        # in the tensor-engine FIFO
                        if s == 0:
                            if (grp + 1) % (CH // GS) == 0:
                                xc = fetch_chunk(grp + 1)
                            P_next = alloc_group()
                        emit_xproj_one(grp + 1, xc, P_next, s)
                    for h in range(2):
                        Ph = P[h]
                        for g in range(4):
                            nc.tensor.matmul(
                                Ph[:, g * GC + s * HB : g * GC + (s + 1) * HB],
                                wh_s[:, g * H : (g + 1) * H],
                                hh[:, h * HB : (h + 1) * HB],
                                start=False,
                                stop=(s == GS - 1 and g == 3),
                            )
                        Wt = W[h]
                        src = Ph.rearrange("p (g s n) -> p g s n", g=4, s=GS)[
                            :, :, s, :
                        ]
                        dst = Wt[:, 32:160].rearrange("p (g n) -> p g n", g=4)
                        nc.scalar.activation(
                            dst, src, mybir.ActivationFunctionType.Sigmoid
                        )
                        # stt1: [r|q] = ([c~|s2g] - 0.5) * [sf|si]
                        i1 = nc.vector.scalar_tensor_tensor(
                            X[h][:],
                            Wt[:, 0:64],
                            0.5,
                            Wt[:, 64:128],
                            ALU.subtract,
                            ALU.mult,
                        )
                        # stt2: c~ = (r + 0.5) + q
                        nc.vector.scalar_tensor_tensor(
                            Wt[:, 0:32],
                            X[h][:, 0:HB],
                            0.5,
                            X[h][:, HB : 2 * HB],
                            ALU.add,
                            ALU.add,
                        )
                        # stt3: hh = (c~ - 0.5) * so
                        i3 = nc.vector.scalar_tensor_tensor(
                            hh[:, h * HB : (h + 1) * HB],
                            Wt[:, 0:32],
                            0.5,
                            Wt[:, 128:160],
                            ALU.subtract,
                            ALU.mult,
                        )
                if P_next is not None:
                    P = P_next

            ypt = ppoolA.tile([128, 4 * GC], FP32, tag="ps")
            yp = ypt[:BL, :C]
            nc.tensor.matmul(yp, hh[:, :], wfc_s[:], start=True, stop=True)
            y_s = cpool.tile([BL, C], FP32)
            nc.vector.tensor_add(y_s[:], yp, bfc_s[:])
            nc.sync.dma_start(y_d.ap(), y_s[:])

    nc.compile()
    return nc


def kernel(x, Wf, bf, Wo, bo, Wi, bi, Wg, bg, Wfc, bfc):
    if "nc" not in _cache:
        _cache["nc"] = _build()
    nc = _cache["nc"]

    # gate order (g, f, i, o); g rows pre-scaled x2 (tanh z = 2*sig(2z)-1);
    # wh doubled because the matmul consumes hh = h/2; wfc likewise doubled.
    gates = [(Wg, bg, 2.0), (Wf, bf, 1.0), (Wi, bi, 1.0), (Wo, bo, 1.0)]
    wx = np.concatenate(
        [
            s * np.concatenate([W[:, :I].T, b[None, :]], axis=0)
            for W, b, s in gates
        ],
        axis=1,
    ).astype(np.float32).astype(jnp_bf16)  # [I+1, 4H]
    wh = np.concatenate(
        [2.0 * s * W[:, I:].T for W, _, s in gates], axis=1
    ).astype(np.float32).astype(jnp_bf16)  # [H, 4H]
    wfc = np.ascontiguousarray(2.0 * Wfc.T).astype(np.float32).astype(jnp_bf16)
    bfcb = np.broadcast_to(bfc, (BL, C)).astype(np.float32).copy()

    in_maps = []
    for cidx in range(NCORES):
        xs = np.asarray(x[cidx * BL : (cidx + 1) * BL], np.float32)  # [BL,T,I]
        xt = np.ascontiguousarray(xs.transpose(2, 1, 0)).reshape(I, T * BL)
        xt = np.concatenate(
            [xt, np.ones((1, T * BL), np.float32)], axis=0
        ).astype(jnp_bf16)
        in_maps.append({"xt": xt, "wx": wx, "wh": wh, "wfc": wfc, "bfcb": bfcb})

    _cache["in_maps"] = in_maps
    res = bass_utils.run_bass_kernel_spmd(
        nc, in_maps, core_ids=list(range(NCORES))
    )
    return np.concatenate([r["y"] for r in res.results], axis=0)
